# revision 1
# baseline (speedup 1.0000x reference)
"""Trainium2 Bass kernel for nn_Attention_64063732187236.

Reference computation (B=4, N=2048, DIM=512, HEADS=8, DIM_HEAD=64):
    qkv = x @ w_qkv ; q,k,v = split(qkv) -> [b,h,n,64]
    atten = softmax((q * HEADS**-0.5) @ k^T + drop_mask * -1e12)
    out   = (atten @ v) reshaped -> [b,n,512] @ w_out

Sharding: 8 cores = 4 batches x 2 head-groups (4 heads each).
Each core computes a partial output  x[b] -> attn(heads hg..hg+3) @ w_out[rows]
and the host sums the two head-group partials per batch.

On-core layout is fully "transposed": scores are computed as
S^T[k,q] = K Q^T so that the PV matmul contracts over k naturally, and the
out-projection consumes attn^T directly.  Softmax skips the max-subtraction
(scores are O(10), exp is safe in fp32) and gets the row-sum for free from a
ones-column appended to V.  Mask is applied post-exp as P *= (1-mask).
"""

import numpy as np

import concourse.bass as bass
import concourse.bacc as bacc
import concourse.tile as tile
from concourse import mybir
from concourse.bass_utils import run_bass_kernel_spmd

F32 = mybir.dt.float32
BF16 = mybir.dt.bfloat16
NP_BF16 = mybir.dt.np(BF16)

# Full-size problem constants
B, N, D = 4, 2048, 512
HEADS, DH = 8, 64
HL = 4               # heads per core (local)
GROUPS = HEADS // HL  # head groups = 2
SCALE = float(HEADS) ** -0.5   # reference quirk: scales by heads, not dim_head


def build_graph(nc, N=N, D=D, HL=HL):
    """Emit the per-core attention graph into `nc` (inside a TileContext)."""
    KT = N // 128          # key tiles
    NT = N // 128          # seq tiles
    DC = D // 128          # d-model chunks
    HP = (HL * DH) // 128  # head-pair chunks (2 for HL=4)
    QW = min(1024, N)      # ACT exp width per op
    NQ = N // QW

    xT = nc.dram_tensor("xT", [D, N], BF16, kind="ExternalInput").ap()
    wqkv = nc.dram_tensor("wqkv", [D, 3 * HL * DH], BF16, kind="ExternalInput").ap()
    wout = nc.dram_tensor("wout", [HL * DH, D], BF16, kind="ExternalInput").ap()
    nmaskT = nc.dram_tensor("nmaskT", [HL, N, N], BF16, kind="ExternalInput").ap()
    out = nc.dram_tensor("out", [N, D], F32, kind="ExternalOutput").ap()
    rscratch = [nc.dram_tensor(f"rscratch{h}", [1, N], F32).ap() for h in range(HL)]

    tc = nc.tc  # set by caller

    with tc.tile_pool(name="wts", bufs=1) as wts, \
         tc.tile_pool(name="persist", bufs=1) as persist:

        # ---- Phase A: inputs arrive pre-cast to bf16; plain DMA loads ----
        xTb = []
        wb = []
        woutb = []
        for dc in range(DC):
            t = wts.tile([128, 3 * HL * DH], BF16, tag=f"wb{dc}", name=f"wb{dc}")
            nc.sync.dma_start(out=t, in_=wqkv[dc * 128:(dc + 1) * 128, :])
            wb.append(t)
        for dc in range(DC):
            t = wts.tile([128, N], BF16, tag=f"xtb{dc}", name=f"xtb{dc}")
            eng = nc.gpsimd if dc % 2 == 0 else nc.sync
            eng.dma_start(out=t, in_=xT[dc * 128:(dc + 1) * 128, :])
            xTb.append(t)
        for c in range(HP):
            t = wts.tile([128, D], BF16, tag=f"wob{c}", name=f"wob{c}")
            nc.sync.dma_start(out=t, in_=wout[c * 128:(c + 1) * 128, :])
            woutb.append(t)

        # ---- Phase B: projections q^T, k^T (head-pair packed) and V(+ones) ----
        qTb = [persist.tile([128, N], BF16, tag=f"qT{p}", name=f"qT{p}") for p in range(HP)]
        kTb = [persist.tile([128, N], BF16, tag=f"kT{p}", name=f"kT{p}") for p in range(HP)]
        vplus = [persist.tile([128, HL, DH + 1], BF16, tag=f"vp{t}", name=f"vp{t}")
                 for t in range(NT)]
        with tc.tile_pool(name="psB", bufs=2, space="PSUM") as psB:
            voff = 2 * HL * DH

            def qk_proj(which, dst, hp, half=None):
                off = which * HL * DH
                halves = range((N + 1023) // 1024) if half is None else [half]
                for half in halves:
                    w = min(1024, N - half * 1024)
                    ps = psB.tile([128, w], F32, tag="qk", name="psqk")
                    for dc in range(DC):
                        for s0 in range(0, w, 512):
                            sw = min(512, w - s0)
                            nc.tensor.matmul(
                                ps[:, s0:s0 + sw],
                                lhsT=wb[dc][:, off + hp * 128: off + (hp + 1) * 128],
                                rhs=xTb[dc][:, half * 1024 + s0: half * 1024 + s0 + sw],
                                start=(dc == 0), stop=(dc == DC - 1))
                    nc.scalar.copy(
                        dst[hp][:, half * 1024: half * 1024 + w], ps)

            # pair 0 first, q/k interleaved by half, so head-0 attention on
            # the first qn half can begin after just two projection rounds
            qk_proj(0, qTb, 0, half=0)
            qk_proj(1, kTb, 0, half=0)
            qk_proj(0, qTb, 0, half=1)
            qk_proj(1, kTb, 0, half=1)
            for nt in range(NT):
                psv = psB.tile([128, HL * DH], F32, tag="v")
                for dc in range(DC):
                    nc.tensor.matmul(
                        psv,
                        lhsT=xTb[dc][:, nt * 128:(nt + 1) * 128],
                        rhs=wb[dc][:, voff: voff + HL * DH],
                        start=(dc == 0), stop=(dc == DC - 1))
                nc.vector.memset(vplus[nt], 1.0)
                nc.vector.tensor_copy(
                    vplus[nt][:, :, 0:DH],
                    psv.rearrange("p (h d) -> p h d", h=HL))
            for hp in range(1, HP):
                qk_proj(0, qTb, hp)
                qk_proj(1, kTb, hp)

        # ---- Phase C: attention per local head + interleaved out-proj ----
        attnT = persist.tile([128, HP, N], BF16, tag="attnT")
        with tc.tile_pool(name="psS", bufs=2, space="PSUM") as psS, \
             tc.tile_pool(name="psPV", bufs=1, space="PSUM") as psPV, \
             tc.tile_pool(name="pmask", bufs=8) as pmask, \
             tc.tile_pool(name="pprob", bufs=9) as pprob, \
             tc.tile_pool(name="psmall", bufs=3) as psmall:
            for h in range(HL):
                hp, ho = h // 2, (h % 2) * 64
                pv = psPV.tile([DH + 1, N], F32, tag="pv")

                def pv_mms(kt, pt):
                    for s0 in range(0, N, 512):
                        sw = min(512, N - s0)
                        nc.tensor.matmul(
                            pv[:, s0:s0 + sw],
                            lhsT=vplus[kt][:, h, :],
                            rhs=pt[:, s0:s0 + sw],
                            start=(kt == 0), stop=(kt == KT - 1))

                # kt loop is software-pipelined: PV matmuls lag one iteration
                # so the PE stream never blocks on the exp/mask of the same kt.
                prev = None
                for kt in range(KT):
                    nm = pmask.tile([128, N], BF16, tag="nm")
                    nc.sync.dma_start(
                        out=nm, in_=nmaskT[h, kt * 128:(kt + 1) * 128, :])
                    if h < HL - 1:
                        pt = pprob.tile([128, N], BF16, tag="pt", name="pt")
                    else:
                        pt = pprob.tile([128, N], BF16, tag="ptl", bufs=3,
                                        name="ptl")
                    with tc.high_priority(offset=150):
                        for qh in range(NQ):
                            s = psS.tile([128, QW], F32, tag="s")
                            for s0 in range(0, QW, 512):
                                sw = min(512, QW - s0)
                                nc.tensor.matmul(
                                    s[:, s0:s0 + sw],
                                    lhsT=kTb[hp][ho:ho + 64, kt * 128:(kt + 1) * 128],
                                    rhs=qTb[hp][ho:ho + 64, qh * QW + s0: qh * QW + s0 + sw],
                                    start=True, stop=True)
                            nc.scalar.activation(
                                pt[:, qh * QW:(qh + 1) * QW], s,
                                mybir.ActivationFunctionType.Exp, scale=SCALE)
                    nc.vector.tensor_mul(pt, pt, nm)
                    if prev is not None:
                        pv_mms(*prev)
                    prev = (kt, pt)
                pv_mms(*prev)
                # Copy pv to SBUF right away (frees PSUM for the next head),
                # then normalize asynchronously: attnT[h] = pv[0:64]/rowsum.
                # rowsum row -> [128, N/128] so reciprocal uses all lanes,
                # then broadcast to 64 partitions via a DRAM-bounce DMA.
                if h < HL - 1:
                    pvs = psmall.tile([DH + 1, N], F32, tag="pvs")
                    nc.vector.tensor_copy(pvs, pv)
                    rsq = psmall.tile([128, N // 128], F32, tag="rsq")
                    nc.sync.dma_start(out=rsq, in_=pvs[DH:DH + 1, :])
                    nc.vector.reciprocal(rsq, rsq)
                    nc.sync.dma_start(out=rscratch[h], in_=rsq)
                    rb = psmall.tile([64, N], F32, tag="rb")
                    rb_src = bass.AP(
                        tensor=rscratch[h].tensor, offset=rscratch[h].offset,
                        ap=[[0, 64]] + list(rscratch[h].ap[1:]))
                    nc.sync.dma_start(out=rb, in_=rb_src)
                    nc.vector.tensor_mul(attnT[ho:ho + 64, hp, :], pvs[0:DH, :], rb)
                else:
                    # last head: normalize + out-projection pipelined by halves
                    # of the sequence so nothing waits on the full-row chain
                    HN = N // 2
                    pvs = psmall.tile([DH + 1, N], F32, tag="pvs")
                    for hh in range(2):
                        hs = slice(hh * HN, (hh + 1) * HN)
                        nc.scalar.copy(pvs[DH:DH + 1, hs], pv[DH:DH + 1, hs])
                        nc.vector.tensor_copy(pvs[0:DH, hs], pv[0:DH, hs])
                        rsq = psmall.tile([128, HN // 128], F32, tag="rsq")
                        nc.sync.dma_start(out=rsq, in_=pvs[DH:DH + 1, hs])
                        nc.vector.reciprocal(rsq, rsq)
                        rsc = rscratch[h][0:1, hs]
                        nc.sync.dma_start(out=rsc, in_=rsq)
                        rb = psmall.tile([64, HN], F32, tag="rb")
                        rb_src = bass.AP(
                            tensor=rsc.tensor, offset=rsc.offset,
                            ap=[[0, 64]] + list(rsc.ap[1:]))
                        nc.sync.dma_start(out=rb, in_=rb_src)
                        NC4 = N // 4
                        for qq in range(2 * hh, 2 * hh + 2):
                            nc.vector.tensor_mul(
                                attnT[ho:ho + 64, hp, qq * NC4:(qq + 1) * NC4],
                                pvs[0:DH, qq * NC4:(qq + 1) * NC4],
                                rb[:, qq * NC4 - hh * HN:(qq + 1) * NC4 - hh * HN])
                            for nt in range(qq * NT // 4, (qq + 1) * NT // 4):
                                po = psS.tile([128, D], F32, tag="s", name="po2")
                                for c in range(HP):
                                    nc.tensor.matmul(
                                        po, lhsT=attnT[:, c, nt * 128:(nt + 1) * 128],
                                        rhs=woutb[c], start=(c == 0),
                                        stop=(c == HP - 1))
                                ob = psmall.tile([128, D], F32, tag="ob", bufs=6)
                                nc.scalar.copy(ob, po)
                                eng = nc.sync if nt % 2 == 0 else nc.gpsimd
                                eng.dma_start(
                                    out=out[nt * 128:(nt + 1) * 128, :], in_=ob)


def build_bass(N=N, D=D, HL=HL):
    nc = bacc.Bacc("TRN2", target_bir_lowering=False, debug=False, num_devices=8)
    with tile.TileContext(nc) as tc:
        nc.tc = tc
        build_graph(nc, N=N, D=D, HL=HL)
    nc.compile()
    return nc


def shard_inputs(x, drop_mask, w_qkv, w_out):
    """Host-side sharding: returns in_maps for the 8 cores."""
    x = np.asarray(x, dtype=np.float32)
    drop_mask = np.asarray(drop_mask)
    w_qkv = np.asarray(w_qkv, dtype=np.float32)
    w_out = np.asarray(w_out, dtype=np.float32)
    inner = HEADS * DH
    in_maps = []
    for c in range(8):
        b, g = c // GROUPS, c % GROUPS
        cols = slice(g * HL * DH, (g + 1) * HL * DH)
        wq = w_qkv[:, cols]
        wk = w_qkv[:, inner:][:, cols]
        wv = w_qkv[:, 2 * inner:][:, cols]
        nmT = np.empty((HL, N, N), dtype=NP_BF16)
        for hh in range(HL):
            nmT[hh] = (~drop_mask[b, g * HL + hh]).T.astype(NP_BF16)
        in_maps.append({
            "xT": np.ascontiguousarray(x[b].T).astype(NP_BF16),
            "wqkv": np.ascontiguousarray(
                np.concatenate([wq, wk, wv], axis=1)).astype(NP_BF16),
            "wout": np.ascontiguousarray(
                w_out[g * HL * DH:(g + 1) * HL * DH, :]).astype(NP_BF16),
            "nmaskT": nmT,
        })
    return in_maps


_CACHED_NC = None


def _get_nc():
    global _CACHED_NC
    if _CACHED_NC is None:
        _CACHED_NC = build_bass()
    return _CACHED_NC


def kernel(x, drop_mask, w_qkv, w_out, _trace=False):
    nc = _get_nc()
    in_maps = shard_inputs(x, drop_mask, w_qkv, w_out)
    res = run_bass_kernel_spmd(nc, in_maps, core_ids=list(range(8)), trace=_trace)
    outs = [np.asarray(r["out"], dtype=np.float32) for r in res.results]
    full = np.empty((B, N, D), dtype=np.float32)
    for b in range(B):
        full[b] = outs[b * GROUPS]
        for g in range(1, GROUPS):
            full[b] += outs[b * GROUPS + g]
    kernel.last_results = res
    return full

